# revision 9
# baseline (speedup 1.0000x reference)
"""Causal attention (B=4, N=2048, D=1024) on 8 Trainium2 NeuronCores.

Sharding: core 2b+p owns batch b's token tiles {p, p+2, ..., p+14}
(parity-interleaved 128-row tiles).  Each core projects Q^T, K^T and V
only for its own 8 tiles (one x load feeds all three projections), then
the two cores of a batch exchange halves in bf16 via two pairwise
AllGathers: first all of K^T (which gates the score matmuls), then all
of V (which only gates the final AV pass).  The collectives overlap
with the V/Q projections and the score phase.

Attention runs over keys in "gather layout" (all parity-0 tiles, then
all parity-1 tiles) so the program is uniform across cores; the causal
masks are per-core input data applied to the tail tile of each parity
region.  Slot i covers i+1 key tiles per region — an exactly balanced
causal split.  Scores for all 8 slots run first (P tiles parked in
SBUF, exp + row-sum fused on the scalar engine, f32 PSUM, single-pass
softmax since |scores|/32 is bounded); the P^T transpose + AV matmuls
run as a second phase once V arrives.

Attention operands (Q^T, K^T, V, P) are bf16: full PE rate at any
moving width, half the exchange volume and SBUF footprint.  Projections
stay float32r.  Score PSUM regions are 1024-aligned so every PSUM bank
has exactly one start=True writer (two starts on one bank corrupt it).
"""
import sys

sys.path.insert(0, "/opt/trn_rl_repo")

from contextlib import ExitStack

import numpy as np

import concourse.bass as bass
import concourse.mybir as mybir
import concourse.tile as tile
from concourse import bacc
from concourse.bass_utils import run_bass_kernel_spmd
from concourse.masks import make_identity

B, N, D = 4, 2048, 1024
N_CORES = 8
N_OWN = 8            # own token tiles per core (q-slots == own k-tiles)
SCALE = 1.0 / 32.0   # 1/sqrt(D)
NEG = -1.0e9
STRIDE = 1024        # per-region column stride in score PSUM / P tiles

F32 = mybir.dt.float32
F32R = mybir.dt.float32r
BF16 = mybir.dt.bfloat16

GROUPS = [[0, 1], [2, 3], [4, 5], [6, 7]]

_NC_CACHE = {}
TRACE = False
LAST_EXEC_NS = None


def _build_nc():
    nc = bacc.Bacc(None, target_bir_lowering=False, debug=False)

    # x for own tiles, d-major: [own_tile, p(d%128), dchunk, token]
    x_own = nc.declare_dram_parameter("x_own", [N_OWN, 128, 8, 128], F32R, isOutput=False)
    # weights host-rearranged: wq/wk [echunk, p(d%128), dchunk, ecol]; wv [eh, p, dchunk, ecol]
    wq = nc.declare_dram_parameter("wq", [8, 128, 8, 128], F32R, isOutput=False)
    wk = nc.declare_dram_parameter("wk", [8, 128, 8, 128], F32R, isOutput=False)
    wv = nc.declare_dram_parameter("wv", [2, 128, 8, 512], F32R, isOutput=False)
    mask_in = nc.declare_dram_parameter("mask", [128, 256], F32, isOutput=False)
    out_q = nc.declare_dram_parameter("out_q", [N_OWN, 128, D], F32, isOutput=True)

    with tile.TileContext(nc) as tc, ExitStack() as top:
        consts = top.enter_context(tc.tile_pool(name="consts", bufs=1))
        kt_pool = top.enter_context(tc.tile_pool(name="ktp", bufs=1))
        v_pool = top.enter_context(tc.tile_pool(name="vp", bufs=1))
        qt_pool = top.enter_context(tc.tile_pool(name="qtp", bufs=1))
        xt_pool = top.enter_context(tc.tile_pool(name="xtp", bufs=1))
        ccdram = top.enter_context(tc.tile_pool(name="ccd", bufs=1, space="DRAM"))

        ident_f = consts.tile([128, 128], F32)
        make_identity(nc, ident_f)
        ident = consts.tile([128, 128], BF16)
        nc.vector.tensor_copy(ident, ident_f)
        mask_sb = consts.tile([128, 256], F32)
        nc.sync.dma_start(out=mask_sb, in_=mask_in[:, :])

        KT = kt_pool.tile([128, 8, N], BF16)       # [p(e%128), echunk, gkey]
        Vt = v_pool.tile([128, 16, D], BF16)       # [p(tok%128), gtile, ecol]
        QT = qt_pool.tile([128, 8, 1024], BF16)    # [p(e%128), echunk, own-q]
        xT = xt_pool.tile([128, N_OWN, 8, 128], F32R)

        # CC bounce: K rows = 2e+g (g = 4-tile group); V rows = 2t+eh
        cin_k = ccdram.tile([16, 128, 512], BF16, name="cin_k")
        cin_v = ccdram.tile([16, 128, 512], BF16, name="cin_v")
        cout_k = ccdram.tile([2, 16, 128, 512], BF16, name="cout_k")
        cout_v = ccdram.tile([2, 16, 128, 512], BF16, name="cout_v")

        for t in range(N_OWN):
            nc.gpsimd.dma_start(out=xT[:, t, :, :], in_=x_own[t][:, :, :])

        with ExitStack() as ph_p:
            wk_pool = ph_p.enter_context(tc.tile_pool(name="wkp", bufs=1))
            wv_pool = ph_p.enter_context(tc.tile_pool(name="wvp", bufs=1))
            stage = ph_p.enter_context(tc.tile_pool(name="stg", bufs=4))
            ps_mm = ph_p.enter_context(tc.tile_pool(name="psmm", bufs=8, space="PSUM"))

            wk_sb = wk_pool.tile([128, 8, 8, 128], F32R)
            for e in range(8):
                nc.scalar.dma_start(out=wk_sb[:, e, :, :], in_=wk[e][:, :, :])
            # chunked wv load: first V matmul only needs c=0..7 of both eh
            wv_sb = wv_pool.tile([128, 2, 8, 512], F32R)
            for c in range(8):
                for eh in range(2):
                    nc.scalar.dma_start(out=wv_sb[:, eh, c, :], in_=wv[eh][:, c, :])

            # ---- K^T for all 8 own tiles, spill, CC_K ----
            for e in range(8):
                for g in range(2):
                    kps = ps_mm.tile([128, 512], F32, tag="mm", name=f"k{e}_{g}")
                    for c in range(8):
                        nc.tensor.matmul(
                            kps, wk_sb[:, e, c, :], xT[:, 4 * g:4 * g + 4, c, :],
                            start=(c == 0), stop=(c == 7),
                        )
                    kst = stage.tile([128, 512], BF16, tag="st", name=f"ks{e}_{g}")
                    nc.vector.tensor_copy(kst, kps)
                    nc.sync.dma_start(out=cin_k[2 * e + g], in_=kst)
            nc.gpsimd.collective_compute(
                "AllGather",
                mybir.AluOpType.bypass,
                replica_groups=GROUPS,
                ins=[cin_k[:, :, :].opt()],
                outs=[cout_k[:, :, :, :].opt()],
            )

            # ---- V for all 8 own tiles, spill, CC_V ----
            for t in range(N_OWN):
                for eh in range(2):
                    vps = ps_mm.tile([128, 512], F32, tag="mm", name=f"v{t}_{eh}")
                    for c in range(8):
                        nc.tensor.matmul(
                            vps, xT[:, t, c, :], wv_sb[:, eh, c, :],
                            start=(c == 0), stop=(c == 7),
                        )
                    vst = stage.tile([128, 512], BF16, tag="st", name=f"vs{t}_{eh}")
                    nc.scalar.activation(vst, vps, mybir.ActivationFunctionType.Copy)
                    nc.sync.dma_start(out=cin_v[2 * t + eh], in_=vst)
            nc.gpsimd.collective_compute(
                "AllGather",
                mybir.AluOpType.bypass,
                replica_groups=GROUPS,
                ins=[cin_v[:, :, :].opt()],
                outs=[cout_v[:, :, :, :].opt()],
            )

            # readback, small striped DMAs; K first (gates scores)
            for r in range(2):
                for e in range(8):
                    for g in range(2):
                        nc.gpsimd.dma_start(
                            out=KT[:, e, r * 1024 + g * 512: r * 1024 + (g + 1) * 512],
                            in_=cout_k[r, 2 * e + g],
                        )
            for t in range(N_OWN):          # tile-major so early AV tiles land first
                for r in range(2):
                    for eh in range(2):
                        nc.gpsimd.dma_start(
                            out=Vt[:, r * 8 + t, eh * 512:(eh + 1) * 512],
                            in_=cout_v[r, 2 * t + eh],
                        )

            # ---- Q^T projections (own tiles == slots), overlap the CC wall
            for e in range(8):
                wq_sb = stage.tile([128, 8, 128], F32R, tag="wq", name=f"wq{e}", bufs=4)
                nc.scalar.dma_start(out=wq_sb, in_=wq[e][:, :, :])
                for qg in range(2):
                    qps = ps_mm.tile([128, 512], F32, tag="mm", name=f"q{e}_{qg}")
                    for c in range(8):
                        nc.tensor.matmul(
                            qps, wq_sb[:, c, :], xT[:, qg * 4:(qg + 1) * 4, c, :],
                            start=(c == 0), stop=(c == 7),
                        )
                    nc.scalar.activation(
                        QT[:, e, qg * 512:(qg + 1) * 512], qps,
                        mybir.ActivationFunctionType.Copy,
                    )

        # ---- attention phase 1: scores + softmax for all slots ----
        with ExitStack() as ph_a:
            p_pool = ph_a.enter_context(tc.tile_pool(name="pp", bufs=1))
            sc_pool = ph_a.enter_context(tc.tile_pool(name="scp", bufs=1))
            pt_pool = ph_a.enter_context(tc.tile_pool(name="ptp", bufs=4))
            outp = ph_a.enter_context(tc.tile_pool(name="outp", bufs=2))

            P_sbs, recips = [], []
            with tc.tile_pool(name="ps_s", bufs=2, space="PSUM") as ps_s:
                for i in range(N_OWN):
                    W = (i + 1) * 128        # per-region score width
                    S_ps = ps_s.tile([128, 2 * STRIDE], F32, tag="S", name=f"S{i}")
                    for e in range(8):
                        for r in range(2):
                            for off in range(0, W, 512):
                                w = min(512, W - off)
                                nc.tensor.matmul(
                                    S_ps[:, r * STRIDE + off: r * STRIDE + off + w],
                                    QT[:, e, i * 128:(i + 1) * 128],
                                    KT[:, e, r * 1024 + off: r * 1024 + off + w],
                                    start=(e == 0), stop=(e == 7),
                                )
                    # causal masks on the tail tile of each region
                    for r in range(2):
                        nc.vector.tensor_add(
                            S_ps[:, r * STRIDE + W - 128: r * STRIDE + W],
                            S_ps[:, r * STRIDE + W - 128: r * STRIDE + W],
                            mask_sb[:, r * 128:(r + 1) * 128],
                        )
                    P_sb = p_pool.tile([128, STRIDE + W], BF16, name=f"P{i}")
                    stats = sc_pool.tile([128, 4], F32, name=f"st{i}")
                    rs = [stats[:, 0:1], stats[:, 1:2]]
                    for r in range(2):
                        nc.scalar.activation(
                            P_sb[:, r * STRIDE: r * STRIDE + W],
                            S_ps[:, r * STRIDE: r * STRIDE + W],
                            mybir.ActivationFunctionType.Exp,
                            bias=0.0, scale=SCALE, accum_out=rs[r],
                        )
                    rowsum = stats[:, 2:3]
                    nc.vector.tensor_add(rowsum, rs[0], rs[1])
                    recip = stats[:, 3:4]
                    nc.vector.reciprocal(recip, rowsum)
                    P_sbs.append(P_sb)
                    recips.append(recip)

            # ---- attention phase 2: P^T transpose + AV ----
            with tc.tile_pool(name="ps_tr", bufs=4, space="PSUM") as ps_tr, \
                 tc.tile_pool(name="ps_o", bufs=2, space="PSUM") as ps_o:
                for i in range(N_OWN):
                    W = (i + 1) * 128
                    npr = i + 1
                    L = 2 * npr
                    O_ps = ps_o.tile([128, D], F32, tag="O", name=f"O{i}")
                    for m in range(L):
                        r, j = divmod(m, npr)
                        g = r * 8 + j        # gather-layout V tile
                        pc = r * STRIDE + j * 128
                        ptps = ps_tr.tile([128, 128], BF16, tag="tr", name=f"tp{i}_{m}")
                        nc.tensor.transpose(ptps, P_sbs[i][:, pc:pc + 128], ident)
                        pt_sb = pt_pool.tile([128, 128], BF16, tag="pts", name=f"pt{i}_{m}")
                        nc.vector.tensor_copy(pt_sb, ptps)
                        for hh in range(2):
                            nc.tensor.matmul(
                                O_ps[:, hh * 512:(hh + 1) * 512], pt_sb,
                                Vt[:, g, hh * 512:(hh + 1) * 512],
                                start=(m == 0), stop=(m == L - 1),
                            )
                    out_sb = outp.tile([128, D], F32, tag="osb", name=f"ou{i}")
                    nc.vector.tensor_scalar_mul(out_sb, O_ps, recips[i])
                    nc.sync.dma_start(out=out_q[i][:, :], in_=out_sb)

    nc.compile()
    return nc


def _masks():
    q = np.arange(128)[:, None]
    k = np.arange(128)[None, :]
    tril_add = np.where(k <= q, 0.0, NEG).astype(np.float32)
    m0 = np.concatenate([tril_add, np.full((128, 128), NEG, np.float32)], axis=1)
    m1 = np.concatenate([np.zeros((128, 128), np.float32), tril_add], axis=1)
    return m0, m1


def kernel(x, Wq, Wk, Wv):
    global LAST_EXEC_NS
    x = np.ascontiguousarray(np.asarray(x, dtype=np.float32))
    Wq = np.ascontiguousarray(np.asarray(Wq, dtype=np.float32))
    Wk = np.ascontiguousarray(np.asarray(Wk, dtype=np.float32))
    Wv = np.ascontiguousarray(np.asarray(Wv, dtype=np.float32))

    if "nc" not in _NC_CACHE:
        _NC_CACHE["nc"] = _build_nc()
    nc = _NC_CACHE["nc"]

    # host pre-transpose: x[b] (N, D) -> (tile, p=d%128, dchunk, token)
    # element (t, p, c, q) = x[b, t*128+q, c*128+p]
    xt_all = np.ascontiguousarray(
        x.reshape(B, 16, 128, 8, 128).transpose(0, 1, 4, 3, 2)
    )  # [B, tile, p, c, q]

    wq_r = np.ascontiguousarray(Wq.reshape(8, 128, 8, 128).transpose(2, 1, 0, 3))
    wk_r = np.ascontiguousarray(Wk.reshape(8, 128, 8, 128).transpose(2, 1, 0, 3))
    wv_r = np.ascontiguousarray(Wv.reshape(8, 128, 2, 512).transpose(2, 1, 0, 3))

    m0, m1 = _masks()
    in_maps = []
    for c in range(N_CORES):
        b, par = divmod(c, 2)
        in_maps.append({
            "x_own": np.ascontiguousarray(xt_all[b, par::2]),
            "wq": wq_r, "wk": wk_r, "wv": wv_r,
            "mask": m1 if par else m0,
        })

    res = run_bass_kernel_spmd(nc, in_maps, list(range(N_CORES)), trace=TRACE)
    LAST_EXEC_NS = res.exec_time_ns

    out = np.empty((B, N, D), dtype=np.float32)
    for c in range(N_CORES):
        b, par = divmod(c, 2)
        oq = res.results[c]["out_q"]
        for i in range(N_OWN):
            g = 2 * i + par
            out[b, g * 128:(g + 1) * 128, :] = oq[i]
    return out


# revision 10
# speedup vs baseline: 1.0503x; 1.0503x over previous
"""Causal attention (B=4, N=2048, D=1024) on 8 Trainium2 NeuronCores.

Sharding: core 2b+p owns batch b's token tiles {p, p+2, ..., p+14}
(parity-interleaved 128-row tiles).  Each core projects Q^T, K^T and V
only for its own 8 tiles (one x load feeds all three projections), then
the two cores of a batch exchange halves in bf16 via two pairwise
AllGathers: first all of K^T (which gates the score matmuls), then all
of V (which only gates the final AV pass).  The collectives overlap
with the V/Q projections and the score phase.

Attention runs over keys in "gather layout" (all parity-0 tiles, then
all parity-1 tiles) so the program is uniform across cores; the causal
masks are per-core input data applied to the tail tile of each parity
region.  Slot i covers i+1 key tiles per region — an exactly balanced
causal split.  Scores for all 8 slots run first (P tiles parked in
SBUF, exp + row-sum fused on the scalar engine, f32 PSUM, single-pass
softmax since |scores|/32 is bounded); the P^T transpose + AV matmuls
run as a second phase once V arrives.

Attention operands (Q^T, K^T, V, P) are bf16: full PE rate at any
moving width, half the exchange volume and SBUF footprint.  Projections
stay float32r.  Score PSUM regions are 1024-aligned so every PSUM bank
has exactly one start=True writer (two starts on one bank corrupt it).
"""
import sys

sys.path.insert(0, "/opt/trn_rl_repo")

from contextlib import ExitStack

import ml_dtypes
import numpy as np

import concourse.bass as bass
import concourse.mybir as mybir
import concourse.tile as tile
from concourse import bacc
from concourse.bass_utils import run_bass_kernel_spmd
from concourse.masks import make_identity

B, N, D = 4, 2048, 1024
N_CORES = 8
N_OWN = 8            # own token tiles per core (q-slots == own k-tiles)
SCALE = 1.0 / 32.0   # 1/sqrt(D)
NEG = -1.0e9
STRIDE = 1024        # per-region column stride in score PSUM / P tiles

F32 = mybir.dt.float32
F32R = mybir.dt.float32r
BF16 = mybir.dt.bfloat16

GROUPS = [[0, 1], [2, 3], [4, 5], [6, 7]]

_NC_CACHE = {}
TRACE = False
LAST_EXEC_NS = None


def _build_nc():
    nc = bacc.Bacc(None, target_bir_lowering=False, debug=False)

    # x for own tiles, d-major: [own_tile, p(d%128), dchunk, token]
    x_own = nc.declare_dram_parameter("x_own", [N_OWN, 128, 8, 128], BF16, isOutput=False)
    # weights host-rearranged: wq/wk [echunk, p(d%128), dchunk, ecol]; wv [eh, p, dchunk, ecol]
    wq = nc.declare_dram_parameter("wq", [8, 128, 8, 128], BF16, isOutput=False)
    wk = nc.declare_dram_parameter("wk", [8, 128, 8, 128], BF16, isOutput=False)
    wv = nc.declare_dram_parameter("wv", [2, 128, 8, 512], BF16, isOutput=False)
    mask_in = nc.declare_dram_parameter("mask", [128, 256], F32, isOutput=False)
    out_q = nc.declare_dram_parameter("out_q", [N_OWN, 128, D], F32, isOutput=True)

    with tile.TileContext(nc) as tc, ExitStack() as top:
        consts = top.enter_context(tc.tile_pool(name="consts", bufs=1))
        kt_pool = top.enter_context(tc.tile_pool(name="ktp", bufs=1))
        v_pool = top.enter_context(tc.tile_pool(name="vp", bufs=1))
        qt_pool = top.enter_context(tc.tile_pool(name="qtp", bufs=1))
        xt_pool = top.enter_context(tc.tile_pool(name="xtp", bufs=1))
        ccdram = top.enter_context(tc.tile_pool(name="ccd", bufs=1, space="DRAM"))

        ident_f = consts.tile([128, 128], F32)
        make_identity(nc, ident_f)
        ident = consts.tile([128, 128], BF16)
        nc.vector.tensor_copy(ident, ident_f)
        mask_sb = consts.tile([128, 256], F32)
        nc.sync.dma_start(out=mask_sb, in_=mask_in[:, :])

        KT = kt_pool.tile([128, 8, N], BF16)       # [p(e%128), echunk, gkey]
        Vt = v_pool.tile([128, 16, D], BF16)       # [p(tok%128), gtile, ecol]
        QT = qt_pool.tile([128, 8, 1024], BF16)    # [p(e%128), echunk, own-q]
        xT = xt_pool.tile([128, N_OWN, 8, 128], BF16)

        # CC bounce: K rows = 2e+g (g = 4-tile group); V rows = 2t+eh
        cin_k = ccdram.tile([16, 128, 512], BF16, name="cin_k")
        cin_v = ccdram.tile([16, 128, 512], BF16, name="cin_v")
        cout_k = ccdram.tile([2, 16, 128, 512], BF16, name="cout_k")
        cout_v = ccdram.tile([2, 16, 128, 512], BF16, name="cout_v")

        for t in range(N_OWN):
            nc.gpsimd.dma_start(out=xT[:, t, :, :], in_=x_own[t][:, :, :])

        with ExitStack() as ph_p:
            wk_pool = ph_p.enter_context(tc.tile_pool(name="wkp", bufs=1))
            wv_pool = ph_p.enter_context(tc.tile_pool(name="wvp", bufs=1))
            stage = ph_p.enter_context(tc.tile_pool(name="stg", bufs=4))
            ps_mm = ph_p.enter_context(tc.tile_pool(name="psmm", bufs=8, space="PSUM"))

            wk_sb = wk_pool.tile([128, 8, 8, 128], BF16)
            for e in range(8):
                nc.scalar.dma_start(out=wk_sb[:, e, :, :], in_=wk[e][:, :, :])
            # chunked wv load: first V matmul only needs c=0..7 of both eh
            wv_sb = wv_pool.tile([128, 2, 8, 512], BF16)
            for c in range(8):
                for eh in range(2):
                    nc.scalar.dma_start(out=wv_sb[:, eh, c, :], in_=wv[eh][:, c, :])

            # ---- K^T for all 8 own tiles, spill, CC_K ----
            for e in range(8):
                for g in range(2):
                    kps = ps_mm.tile([128, 512], F32, tag="mm", name=f"k{e}_{g}")
                    for c in range(8):
                        nc.tensor.matmul(
                            kps, wk_sb[:, e, c, :], xT[:, 4 * g:4 * g + 4, c, :],
                            start=(c == 0), stop=(c == 7),
                        )
                    kst = stage.tile([128, 512], BF16, tag="st", name=f"ks{e}_{g}")
                    nc.vector.tensor_copy(kst, kps)
                    nc.sync.dma_start(out=cin_k[2 * e + g], in_=kst)
            nc.gpsimd.collective_compute(
                "AllGather",
                mybir.AluOpType.bypass,
                replica_groups=GROUPS,
                ins=[cin_k[:, :, :].opt()],
                outs=[cout_k[:, :, :, :].opt()],
            )

            # ---- V for all 8 own tiles, spill, CC_V ----
            for t in range(N_OWN):
                for eh in range(2):
                    vps = ps_mm.tile([128, 512], F32, tag="mm", name=f"v{t}_{eh}")
                    for c in range(8):
                        nc.tensor.matmul(
                            vps, xT[:, t, c, :], wv_sb[:, eh, c, :],
                            start=(c == 0), stop=(c == 7),
                        )
                    vst = stage.tile([128, 512], BF16, tag="st", name=f"vs{t}_{eh}")
                    nc.scalar.activation(vst, vps, mybir.ActivationFunctionType.Copy)
                    nc.sync.dma_start(out=cin_v[2 * t + eh], in_=vst)
            nc.gpsimd.collective_compute(
                "AllGather",
                mybir.AluOpType.bypass,
                replica_groups=GROUPS,
                ins=[cin_v[:, :, :].opt()],
                outs=[cout_v[:, :, :, :].opt()],
            )

            # readback, small striped DMAs; K first (gates scores)
            for r in range(2):
                for e in range(8):
                    for g in range(2):
                        nc.gpsimd.dma_start(
                            out=KT[:, e, r * 1024 + g * 512: r * 1024 + (g + 1) * 512],
                            in_=cout_k[r, 2 * e + g],
                        )
            for t in range(N_OWN):          # tile-major so early AV tiles land first
                for r in range(2):
                    for eh in range(2):
                        nc.gpsimd.dma_start(
                            out=Vt[:, r * 8 + t, eh * 512:(eh + 1) * 512],
                            in_=cout_v[r, 2 * t + eh],
                        )

            # ---- Q^T projections (own tiles == slots), overlap the CC wall
            for e in range(8):
                wq_sb = stage.tile([128, 8, 128], BF16, tag="wq", name=f"wq{e}", bufs=4)
                nc.scalar.dma_start(out=wq_sb, in_=wq[e][:, :, :])
                for qg in range(2):
                    qps = ps_mm.tile([128, 512], F32, tag="mm", name=f"q{e}_{qg}")
                    for c in range(8):
                        nc.tensor.matmul(
                            qps, wq_sb[:, c, :], xT[:, qg * 4:(qg + 1) * 4, c, :],
                            start=(c == 0), stop=(c == 7),
                        )
                    nc.scalar.activation(
                        QT[:, e, qg * 512:(qg + 1) * 512], qps,
                        mybir.ActivationFunctionType.Copy,
                    )

        # ---- attention phase 1: scores + softmax for all slots ----
        with ExitStack() as ph_a:
            p_pool = ph_a.enter_context(tc.tile_pool(name="pp", bufs=1))
            sc_pool = ph_a.enter_context(tc.tile_pool(name="scp", bufs=1))
            pt_pool = ph_a.enter_context(tc.tile_pool(name="ptp", bufs=4))
            outp = ph_a.enter_context(tc.tile_pool(name="outp", bufs=2))

            P_sbs, recips = [], []
            with tc.tile_pool(name="ps_s", bufs=2, space="PSUM") as ps_s:
                for i in range(N_OWN):
                    W = (i + 1) * 128        # per-region score width
                    S_ps = ps_s.tile([128, 2 * STRIDE], F32, tag="S", name=f"S{i}")
                    for e in range(8):
                        for r in range(2):
                            for off in range(0, W, 512):
                                w = min(512, W - off)
                                nc.tensor.matmul(
                                    S_ps[:, r * STRIDE + off: r * STRIDE + off + w],
                                    QT[:, e, i * 128:(i + 1) * 128],
                                    KT[:, e, r * 1024 + off: r * 1024 + off + w],
                                    start=(e == 0), stop=(e == 7),
                                )
                    # causal masks on the tail tile of each region
                    for r in range(2):
                        nc.vector.tensor_add(
                            S_ps[:, r * STRIDE + W - 128: r * STRIDE + W],
                            S_ps[:, r * STRIDE + W - 128: r * STRIDE + W],
                            mask_sb[:, r * 128:(r + 1) * 128],
                        )
                    P_sb = p_pool.tile([128, STRIDE + W], BF16, name=f"P{i}")
                    stats = sc_pool.tile([128, 4], F32, name=f"st{i}")
                    rs = [stats[:, 0:1], stats[:, 1:2]]
                    for r in range(2):
                        nc.scalar.activation(
                            P_sb[:, r * STRIDE: r * STRIDE + W],
                            S_ps[:, r * STRIDE: r * STRIDE + W],
                            mybir.ActivationFunctionType.Exp,
                            bias=0.0, scale=SCALE, accum_out=rs[r],
                        )
                    rowsum = stats[:, 2:3]
                    nc.vector.tensor_add(rowsum, rs[0], rs[1])
                    recip = stats[:, 3:4]
                    nc.vector.reciprocal(recip, rowsum)
                    P_sbs.append(P_sb)
                    recips.append(recip)

            # ---- attention phase 2: P^T transpose + AV ----
            with tc.tile_pool(name="ps_tr", bufs=4, space="PSUM") as ps_tr, \
                 tc.tile_pool(name="ps_o", bufs=2, space="PSUM") as ps_o:
                for i in range(N_OWN):
                    W = (i + 1) * 128
                    npr = i + 1
                    L = 2 * npr
                    O_ps = ps_o.tile([128, D], F32, tag="O", name=f"O{i}")
                    for m in range(L):
                        r, j = divmod(m, npr)
                        g = r * 8 + j        # gather-layout V tile
                        pc = r * STRIDE + j * 128
                        ptps = ps_tr.tile([128, 128], BF16, tag="tr", name=f"tp{i}_{m}")
                        nc.tensor.transpose(ptps, P_sbs[i][:, pc:pc + 128], ident)
                        pt_sb = pt_pool.tile([128, 128], BF16, tag="pts", name=f"pt{i}_{m}")
                        nc.vector.tensor_copy(pt_sb, ptps)
                        for hh in range(2):
                            nc.tensor.matmul(
                                O_ps[:, hh * 512:(hh + 1) * 512], pt_sb,
                                Vt[:, g, hh * 512:(hh + 1) * 512],
                                start=(m == 0), stop=(m == L - 1),
                            )
                    out_sb = outp.tile([128, D], F32, tag="osb", name=f"ou{i}")
                    nc.vector.tensor_scalar_mul(out_sb, O_ps, recips[i])
                    nc.sync.dma_start(out=out_q[i][:, :], in_=out_sb)

    nc.compile()
    return nc


def _masks():
    q = np.arange(128)[:, None]
    k = np.arange(128)[None, :]
    tril_add = np.where(k <= q, 0.0, NEG).astype(np.float32)
    m0 = np.concatenate([tril_add, np.full((128, 128), NEG, np.float32)], axis=1)
    m1 = np.concatenate([np.zeros((128, 128), np.float32), tril_add], axis=1)
    return m0, m1


def kernel(x, Wq, Wk, Wv):
    global LAST_EXEC_NS
    x = np.ascontiguousarray(np.asarray(x, dtype=np.float32))
    Wq = np.ascontiguousarray(np.asarray(Wq, dtype=np.float32))
    Wk = np.ascontiguousarray(np.asarray(Wk, dtype=np.float32))
    Wv = np.ascontiguousarray(np.asarray(Wv, dtype=np.float32))

    if "nc" not in _NC_CACHE:
        _NC_CACHE["nc"] = _build_nc()
    nc = _NC_CACHE["nc"]

    # host pre-transpose: x[b] (N, D) -> (tile, p=d%128, dchunk, token)
    # element (t, p, c, q) = x[b, t*128+q, c*128+p]
    xt_all = np.ascontiguousarray(
        x.reshape(B, 16, 128, 8, 128).transpose(0, 1, 4, 3, 2)
    ).astype(ml_dtypes.bfloat16)  # [B, tile, p, c, q]

    wq_r = np.ascontiguousarray(Wq.reshape(8, 128, 8, 128).transpose(2, 1, 0, 3)).astype(ml_dtypes.bfloat16)
    wk_r = np.ascontiguousarray(Wk.reshape(8, 128, 8, 128).transpose(2, 1, 0, 3)).astype(ml_dtypes.bfloat16)
    wv_r = np.ascontiguousarray(Wv.reshape(8, 128, 2, 512).transpose(2, 1, 0, 3)).astype(ml_dtypes.bfloat16)

    m0, m1 = _masks()
    in_maps = []
    for c in range(N_CORES):
        b, par = divmod(c, 2)
        in_maps.append({
            "x_own": np.ascontiguousarray(xt_all[b, par::2]),
            "wq": wq_r, "wk": wk_r, "wv": wv_r,
            "mask": m1 if par else m0,
        })

    res = run_bass_kernel_spmd(nc, in_maps, list(range(N_CORES)), trace=TRACE)
    LAST_EXEC_NS = res.exec_time_ns

    out = np.empty((B, N, D), dtype=np.float32)
    for c in range(N_CORES):
        b, par = divmod(c, 2)
        oq = res.results[c]["out_q"]
        for i in range(N_OWN):
            g = 2 * i + par
            out[b, g * 128:(g + 1) * 128, :] = oq[i]
    return out


# revision 12
# speedup vs baseline: 1.0738x; 1.0224x over previous
"""Causal attention (B=4, N=2048, D=1024) on 8 Trainium2 NeuronCores.

Sharding: core 2b+p owns batch b's token tiles {p, p+2, ..., p+14}
(parity-interleaved 128-row tiles).  Each core projects Q^T, K^T and V
only for its own 8 tiles (one x load feeds all three projections), then
the two cores of a batch exchange halves in bf16 via two pairwise
AllGathers: first all of K^T (which gates the score matmuls), then all
of V (which only gates the final AV pass).  The collectives overlap
with the V/Q projections and the score phase.

Attention runs over keys in "gather layout" (all parity-0 tiles, then
all parity-1 tiles) so the program is uniform across cores; the causal
masks are per-core input data applied to the tail tile of each parity
region.  Slot i covers i+1 key tiles per region — an exactly balanced
causal split.  Scores for all 8 slots run first (P tiles parked in
SBUF, exp + row-sum fused on the scalar engine, f32 PSUM, single-pass
softmax since |scores|/32 is bounded); the P^T transpose + AV matmuls
run as a second phase once V arrives.

Attention operands (Q^T, K^T, V, P) are bf16: full PE rate at any
moving width, half the exchange volume and SBUF footprint.  Projections
stay float32r.  Score PSUM regions are 1024-aligned so every PSUM bank
has exactly one start=True writer (two starts on one bank corrupt it).
"""
import sys

sys.path.insert(0, "/opt/trn_rl_repo")

from contextlib import ExitStack

import ml_dtypes
import numpy as np

import concourse.bass as bass
import concourse.mybir as mybir
import concourse.tile as tile
from concourse import bacc
from concourse.bass_utils import run_bass_kernel_spmd
from concourse.masks import make_identity

B, N, D = 4, 2048, 1024
N_CORES = 8
N_OWN = 8            # own token tiles per core (q-slots == own k-tiles)
SCALE = 1.0 / 32.0   # 1/sqrt(D)
NEG = -1.0e9
STRIDE = 1024        # per-region column stride in score PSUM / P tiles

F32 = mybir.dt.float32
F32R = mybir.dt.float32r
BF16 = mybir.dt.bfloat16

GROUPS = [[0, 1], [2, 3], [4, 5], [6, 7]]

_NC_CACHE = {}
TRACE = False
LAST_EXEC_NS = None


def _build_nc():
    nc = bacc.Bacc(None, target_bir_lowering=False, debug=False)

    # x for own tiles, d-major: [own_tile, p(d%128), dchunk, token]
    x_own = nc.declare_dram_parameter("x_own", [N_OWN, 128, 8, 128], BF16, isOutput=False)
    # weights host-rearranged: wq/wk [echunk, p(d%128), dchunk, ecol]; wv [eh, p, dchunk, ecol]
    wq = nc.declare_dram_parameter("wq", [8, 128, 8, 128], BF16, isOutput=False)
    wk = nc.declare_dram_parameter("wk", [8, 128, 8, 128], BF16, isOutput=False)
    wv = nc.declare_dram_parameter("wv", [2, 128, 8, 512], BF16, isOutput=False)
    mask_in = nc.declare_dram_parameter("mask", [128, 256], F32, isOutput=False)
    out_q = nc.declare_dram_parameter("out_q", [N_OWN, 128, D], F32, isOutput=True)

    with tile.TileContext(nc) as tc, ExitStack() as top:
        consts = top.enter_context(tc.tile_pool(name="consts", bufs=1))
        kt_pool = top.enter_context(tc.tile_pool(name="ktp", bufs=1))
        v_pool = top.enter_context(tc.tile_pool(name="vp", bufs=1))
        qt_pool = top.enter_context(tc.tile_pool(name="qtp", bufs=1))
        xt_pool = top.enter_context(tc.tile_pool(name="xtp", bufs=1))
        ccdram = top.enter_context(tc.tile_pool(name="ccd", bufs=1, space="DRAM"))

        ident_f = consts.tile([128, 128], F32)
        make_identity(nc, ident_f)
        ident = consts.tile([128, 128], BF16)
        nc.vector.tensor_copy(ident, ident_f)
        mask_sb = consts.tile([128, 256], F32)
        nc.sync.dma_start(out=mask_sb, in_=mask_in[:, :])

        KT = kt_pool.tile([128, 8, N], BF16)       # [p(e%128), echunk, gkey]
        Vt = v_pool.tile([128, 16, D], BF16)       # [p(tok%128), gtile, ecol]
        QT = qt_pool.tile([128, 8, 1024], BF16)    # [p(e%128), echunk, own-q]
        xT = xt_pool.tile([128, N_OWN, 8, 128], BF16)

        # CC bounce: K rows = 2e+g (g = 4-tile group); V rows = 2t+eh
        cin_k = ccdram.tile([16, 128, 512], BF16, name="cin_k")
        cin_v = ccdram.tile([16, 128, 512], BF16, name="cin_v")
        cout_k = ccdram.tile([2, 16, 128, 512], BF16, name="cout_k")
        cout_v = ccdram.tile([2, 16, 128, 512], BF16, name="cout_v")

        for t in range(N_OWN):
            nc.gpsimd.dma_start(out=xT[:, t, :, :], in_=x_own[t][:, :, :])

        with ExitStack() as ph_p:
            wk_pool = ph_p.enter_context(tc.tile_pool(name="wkp", bufs=1))
            wv_pool = ph_p.enter_context(tc.tile_pool(name="wvp", bufs=1))
            stage = ph_p.enter_context(tc.tile_pool(name="stg", bufs=4))
            ps_mm = ph_p.enter_context(tc.tile_pool(name="psmm", bufs=8, space="PSUM"))

            wk_sb = wk_pool.tile([128, 8, 8, 128], BF16)
            for e in range(8):
                nc.scalar.dma_start(out=wk_sb[:, e, :, :], in_=wk[e][:, :, :])
            # chunked wv load: first V matmul only needs c=0..7 of both eh
            wv_sb = wv_pool.tile([128, 2, 8, 512], BF16)
            for c in range(8):
                for eh in range(2):
                    nc.scalar.dma_start(out=wv_sb[:, eh, c, :], in_=wv[eh][:, c, :])

            # ---- K^T for all 8 own tiles, spill, CC_K ----
            for e in range(8):
                for g in range(2):
                    kps = ps_mm.tile([128, 512], F32, tag="mm", name=f"k{e}_{g}")
                    for c in range(8):
                        nc.tensor.matmul(
                            kps, wk_sb[:, e, c, :], xT[:, 4 * g:4 * g + 4, c, :],
                            start=(c == 0), stop=(c == 7),
                        )
                    kst = stage.tile([128, 512], BF16, tag="st", name=f"ks{e}_{g}")
                    nc.vector.tensor_copy(kst, kps)
                    nc.sync.dma_start(out=cin_k[2 * e + g], in_=kst)
            nc.gpsimd.collective_compute(
                "AllGather",
                mybir.AluOpType.bypass,
                replica_groups=GROUPS,
                ins=[cin_k[:, :, :].opt()],
                outs=[cout_k[:, :, :, :].opt()],
            )

            # ---- V for all 8 own tiles, spill, CC_V ----
            for t in range(N_OWN):
                for eh in range(2):
                    vps = ps_mm.tile([128, 512], F32, tag="mm", name=f"v{t}_{eh}")
                    for c in range(8):
                        nc.tensor.matmul(
                            vps, xT[:, t, c, :], wv_sb[:, eh, c, :],
                            start=(c == 0), stop=(c == 7),
                        )
                    vst = stage.tile([128, 512], BF16, tag="st", name=f"vs{t}_{eh}")
                    nc.scalar.activation(vst, vps, mybir.ActivationFunctionType.Copy)
                    nc.sync.dma_start(out=cin_v[2 * t + eh], in_=vst)
            nc.gpsimd.collective_compute(
                "AllGather",
                mybir.AluOpType.bypass,
                replica_groups=GROUPS,
                ins=[cin_v[:, :, :].opt()],
                outs=[cout_v[:, :, :, :].opt()],
            )

            # readback, small DMAs spread over the three DMA-capable queues;
            # K g=0 first (gates early score slots), then g=1, then V
            # tile-major (so early AV tiles land first)
            for r in range(2):
                for e in range(8):
                    eng = nc.sync if e % 2 == 0 else nc.scalar
                    eng.dma_start(
                        out=KT[:, e, r * 1024: r * 1024 + 512],
                        in_=cout_k[r, 2 * e],
                    )
            for r in range(2):
                for e in range(8):
                    nc.gpsimd.dma_start(
                        out=KT[:, e, r * 1024 + 512: r * 1024 + 1024],
                        in_=cout_k[r, 2 * e + 1],
                    )
            for t in range(N_OWN):
                for r in range(2):
                    for eh in range(2):
                        eng = nc.sync if (r + eh) % 2 == 0 else nc.gpsimd
                        eng.dma_start(
                            out=Vt[:, r * 8 + t, eh * 512:(eh + 1) * 512],
                            in_=cout_v[r, 2 * t + eh],
                        )

            # ---- Q^T projections (own tiles == slots), overlap the CC wall
            for e in range(8):
                wq_sb = stage.tile([128, 8, 128], BF16, tag="wq", name=f"wq{e}", bufs=4)
                nc.scalar.dma_start(out=wq_sb, in_=wq[e][:, :, :])
                for qg in range(2):
                    qps = ps_mm.tile([128, 512], F32, tag="mm", name=f"q{e}_{qg}")
                    for c in range(8):
                        nc.tensor.matmul(
                            qps, wq_sb[:, c, :], xT[:, qg * 4:(qg + 1) * 4, c, :],
                            start=(c == 0), stop=(c == 7),
                        )
                    nc.scalar.activation(
                        QT[:, e, qg * 512:(qg + 1) * 512], qps,
                        mybir.ActivationFunctionType.Copy,
                    )

        # ---- attention phase 1: scores + softmax for all slots ----
        with ExitStack() as ph_a:
            p_pool = ph_a.enter_context(tc.tile_pool(name="pp", bufs=1))
            sc_pool = ph_a.enter_context(tc.tile_pool(name="scp", bufs=1))
            pt_pool = ph_a.enter_context(tc.tile_pool(name="ptp", bufs=4))
            outp = ph_a.enter_context(tc.tile_pool(name="outp", bufs=2))

            P_sbs, recips = [], []
            with tc.tile_pool(name="ps_s", bufs=2, space="PSUM") as ps_s:
                for i in range(N_OWN):
                    W = (i + 1) * 128        # per-region score width
                    S_ps = ps_s.tile([128, 2 * STRIDE], F32, tag="S", name=f"S{i}")
                    for e in range(8):
                        for r in range(2):
                            for off in range(0, W, 512):
                                w = min(512, W - off)
                                nc.tensor.matmul(
                                    S_ps[:, r * STRIDE + off: r * STRIDE + off + w],
                                    QT[:, e, i * 128:(i + 1) * 128],
                                    KT[:, e, r * 1024 + off: r * 1024 + off + w],
                                    start=(e == 0), stop=(e == 7),
                                )
                    # causal masks on the tail tile of each region
                    for r in range(2):
                        nc.vector.tensor_add(
                            S_ps[:, r * STRIDE + W - 128: r * STRIDE + W],
                            S_ps[:, r * STRIDE + W - 128: r * STRIDE + W],
                            mask_sb[:, r * 128:(r + 1) * 128],
                        )
                    P_sb = p_pool.tile([128, STRIDE + W], BF16, name=f"P{i}")
                    stats = sc_pool.tile([128, 4], F32, name=f"st{i}")
                    rs = [stats[:, 0:1], stats[:, 1:2]]
                    for r in range(2):
                        nc.scalar.activation(
                            P_sb[:, r * STRIDE: r * STRIDE + W],
                            S_ps[:, r * STRIDE: r * STRIDE + W],
                            mybir.ActivationFunctionType.Exp,
                            bias=0.0, scale=SCALE, accum_out=rs[r],
                        )
                    rowsum = stats[:, 2:3]
                    nc.vector.tensor_add(rowsum, rs[0], rs[1])
                    recip = stats[:, 3:4]
                    nc.vector.reciprocal(recip, rowsum)
                    P_sbs.append(P_sb)
                    recips.append(recip)

            # ---- attention phase 2: P^T transpose + AV ----
            # all transposes of a slot first, AV matmuls after: hides the
            # PE->DVE->PE round-trip latency of the pt copy per key tile
            with tc.tile_pool(name="ps_tr", bufs=4, space="PSUM") as ps_tr, \
                 tc.tile_pool(name="ps_o", bufs=2, space="PSUM") as ps_o:
                for i in range(N_OWN):
                    npr = i + 1
                    L = 2 * npr
                    O_ps = ps_o.tile([128, D], F32, tag="O", name=f"O{i}")
                    pts = []
                    for m in range(L):
                        r, j = divmod(m, npr)
                        pc = r * STRIDE + j * 128
                        ptps = ps_tr.tile([128, 128], BF16, tag="tr", name=f"tp{i}_{m}")
                        nc.tensor.transpose(ptps, P_sbs[i][:, pc:pc + 128], ident)
                        pt_sb = pt_pool.tile([128, 128], BF16, tag="pts",
                                             name=f"pt{i}_{m}", bufs=18)
                        nc.vector.tensor_copy(pt_sb, ptps)
                        pts.append(pt_sb)
                    for m in range(L):
                        r, j = divmod(m, npr)
                        g = r * 8 + j        # gather-layout V tile
                        for hh in range(2):
                            nc.tensor.matmul(
                                O_ps[:, hh * 512:(hh + 1) * 512], pts[m],
                                Vt[:, g, hh * 512:(hh + 1) * 512],
                                start=(m == 0), stop=(m == L - 1),
                            )
                    out_sb = outp.tile([128, D], F32, tag="osb", name=f"ou{i}")
                    nc.vector.tensor_scalar_mul(out_sb, O_ps, recips[i])
                    nc.sync.dma_start(out=out_q[i][:, :], in_=out_sb)

    nc.compile()
    return nc


def _masks():
    q = np.arange(128)[:, None]
    k = np.arange(128)[None, :]
    tril_add = np.where(k <= q, 0.0, NEG).astype(np.float32)
    m0 = np.concatenate([tril_add, np.full((128, 128), NEG, np.float32)], axis=1)
    m1 = np.concatenate([np.zeros((128, 128), np.float32), tril_add], axis=1)
    return m0, m1


def kernel(x, Wq, Wk, Wv):
    global LAST_EXEC_NS
    x = np.ascontiguousarray(np.asarray(x, dtype=np.float32))
    Wq = np.ascontiguousarray(np.asarray(Wq, dtype=np.float32))
    Wk = np.ascontiguousarray(np.asarray(Wk, dtype=np.float32))
    Wv = np.ascontiguousarray(np.asarray(Wv, dtype=np.float32))

    if "nc" not in _NC_CACHE:
        _NC_CACHE["nc"] = _build_nc()
    nc = _NC_CACHE["nc"]

    # host pre-transpose: x[b] (N, D) -> (tile, p=d%128, dchunk, token)
    # element (t, p, c, q) = x[b, t*128+q, c*128+p]
    xt_all = np.ascontiguousarray(
        x.reshape(B, 16, 128, 8, 128).transpose(0, 1, 4, 3, 2)
    ).astype(ml_dtypes.bfloat16)  # [B, tile, p, c, q]

    wq_r = np.ascontiguousarray(Wq.reshape(8, 128, 8, 128).transpose(2, 1, 0, 3)).astype(ml_dtypes.bfloat16)
    wk_r = np.ascontiguousarray(Wk.reshape(8, 128, 8, 128).transpose(2, 1, 0, 3)).astype(ml_dtypes.bfloat16)
    wv_r = np.ascontiguousarray(Wv.reshape(8, 128, 2, 512).transpose(2, 1, 0, 3)).astype(ml_dtypes.bfloat16)

    m0, m1 = _masks()
    in_maps = []
    for c in range(N_CORES):
        b, par = divmod(c, 2)
        in_maps.append({
            "x_own": np.ascontiguousarray(xt_all[b, par::2]),
            "wq": wq_r, "wk": wk_r, "wv": wv_r,
            "mask": m1 if par else m0,
        })

    res = run_bass_kernel_spmd(nc, in_maps, list(range(N_CORES)), trace=TRACE)
    LAST_EXEC_NS = res.exec_time_ns

    out = np.empty((B, N, D), dtype=np.float32)
    for c in range(N_CORES):
        b, par = divmod(c, 2)
        oq = res.results[c]["out_q"]
        for i in range(N_OWN):
            g = 2 * i + par
            out[b, g * 128:(g + 1) * 128, :] = oq[i]
    return out
